# revision 35
# baseline (speedup 1.0000x reference)
"""Anisotropic distance kernel for 8 Trainium2 NeuronCores.

Math: the reference computes, per batch b and point pair (i, j):

    along  = (p_i - p_j) . t_i
    sqd    = ||p_i - p_j||^2           (gram trick)
    out    = alpha_i * max(sqd + (||t_i||^2 - 2) * along^2, 0) + beta_i * along^2

With g_i = alpha_i*(||t_i||^2 - 2) + beta_i and h_i = alpha_i this is (up to the
clamp, whose argument is nonnegative mathematically — it only clips ~1e-6
rounding noise):

    out[i, j] = g_i * along^2 + h_i * sqd

Both along^2 and sqd are quadratic polynomials in p_j, so out[i, j] is a dot
product of a 10-vector of i-coefficients with the fixed monomial basis
    G(p_j) = [p1^2, p2^2, p3^2, p1 p2, p1 p3, p2 p3, p1, p2, p3, 1].

The entire N x N map is therefore ONE matmul with contraction K=10. It runs on
the TensorEngine in bf16 with a hi/lo split (K=30) for ~fp32 accuracy at full
PE speed: sum = L_hi.G_hi + L_lo.G_hi + L_hi.G_lo (error ~2^-18 per term).
Since K=30 <= 32, the features are duplicated on two partition strips (rows
0-29 and 32-61) and consecutive matmuls alternate tile_position row groups,
so pairs of matmuls run concurrently in independent 32-row strips of the PE
array (row tiling) — the matmul supply outruns the DMA drain even cold.

Sharding: output rows i are split across the 8 cores (1024 rows/core/batch);
the 30-row j-basis G is replicated. Per core: 256 matmuls [128x512] (weights
change once per 16 matmuls), 1MB PSUM->SBUF copies alternating the scalar and
vector engines, and 4MB-per-row-block output DMAs. The kernel is bound by the
output write stream: 64MB/core through 16 SDMA engines at ~27 GB/s each
(~430 GB/s aggregate); a short junk-matmul warmup latches the PE HAM clock
gate so the matmul supply keeps the DMA queue fed from the first tile.
"""

import numpy as np
import ml_dtypes

import concourse.bass as bass  # noqa: F401  (engine namespaces live on nc)
import concourse.mybir as mybir
from concourse import bacc, tile
from concourse.bass_utils import run_bass_kernel_spmd

N_CORES = 8
KF = 30            # contraction: 10 basis monomials x 3 hi/lo cross terms
FD = 512           # matmul moving free dim (one fp32 PSUM bank)
PC = 2048          # PSUM tile columns (4 banks) per copy instruction
N_WARM = 16        # junk matmuls at t=0 to latch the PE HAM clock gate

_BF16 = ml_dtypes.bfloat16
_NC_CACHE = {}


def _features(points, principal_dir, linearity):
    """Host-side O(N) prep: the 10-row j-basis G and i-coefficients L."""
    p = points.astype(np.float64)
    t = principal_dir.astype(np.float64)
    lin = linearity.astype(np.float64)[..., 0]
    p1, p2, p3 = p[..., 0], p[..., 1], p[..., 2]
    t1, t2, t3 = t[..., 0], t[..., 1], t[..., 2]
    one = np.ones_like(p1)
    G = np.stack(
        [p1 * p1, p2 * p2, p3 * p3, p1 * p2, p1 * p3, p2 * p3, p1, p2, p3, one],
        axis=1,
    )  # [B, 10, N]
    s = (t * p).sum(-1)
    sq = (p * p).sum(-1)
    tn2 = (t * t).sum(-1)
    alpha = 2.0 * (1.0 + lin)
    beta = 0.5 * (1.0 - lin)
    g = alpha * (tn2 - 2.0) + beta
    h = alpha
    L = np.stack(
        [
            g * t1 * t1 + h,
            g * t2 * t2 + h,
            g * t3 * t3 + h,
            2.0 * g * t1 * t2,
            2.0 * g * t1 * t3,
            2.0 * g * t2 * t3,
            -2.0 * (g * s * t1 + h * p1),
            -2.0 * (g * s * t2 + h * p2),
            -2.0 * (g * s * t3 + h * p3),
            g * s * s + h * sq,
        ],
        axis=1,
    )  # [B, 10, N]
    return G, L


def _hilo(x):
    x32 = x.astype(np.float32)
    hi = x32.astype(_BF16)
    lo = (x32 - hi.astype(np.float32)).astype(_BF16)
    return hi, lo


def _build(b_sz, n_sz, s_sz):
    key = (b_sz, n_sz, s_sz)
    if key in _NC_CACHE:
        return _NC_CACHE[key]
    nc = bacc.Bacc("TRN2", debug=False, target_bir_lowering=False)
    g_ext = nc.declare_dram_parameter(
        "gfeat", [b_sz, 4, KF, n_sz // 4], mybir.dt.bfloat16, isOutput=False
    )
    l_ext = nc.declare_dram_parameter(
        "lfeat", [128, b_sz * s_sz], mybir.dt.bfloat16, isOutput=False
    )
    out_ext = nc.declare_dram_parameter(
        "out", [b_sz, s_sz, n_sz], mybir.dt.float32, isOutput=True
    )
    pc = min(PC, n_sz // 4)   # one PSUM tile spans one column-quarter (strip)
    fd = min(FD, pc)
    n_it = s_sz // 128
    copy_idx = 0
    with tile.TileContext(nc) as tc:
        with (
            tc.tile_pool(name="gpool", bufs=2) as gp,
            tc.tile_pool(name="lpool", bufs=1) as lp,
            tc.tile_pool(name="opool", bufs=4) as op,
            tc.tile_pool(name="ppool", bufs=2, space="PSUM") as pp,
        ):
            # Input loads: L first (tiny, gates everything), then the basis
            # quarter-striped: strip q (partitions 32q..32q+29) holds only
            # quarter-q's columns, so the four loads hit disjoint SDMA
            # engine sets and run concurrently. No replication, no SWDGE,
            # no PE warmup: the row-tiled matmul quads outrun the DMA drain
            # even at the cold clock.
            lt = lp.tile([128, b_sz * s_sz], mybir.dt.bfloat16)
            nc.sync.dma_start(lt[:], l_ext.ap())
            quarter = n_sz // 4
            gts = []
            for b in range(b_sz):
                gt = gp.tile([128, quarter], mybir.dt.bfloat16, tag="g")
                for k in range(4):
                    eng = nc.sync if k % 2 == 0 else nc.scalar
                    eng.dma_start(
                        gt[32 * k : 32 * k + KF, :],
                        g_ext.ap()[b, k],
                    )
                gts.append(gt)
            for b in range(b_sz):
                for it in range(n_it):
                    i0 = b * s_sz + it * 128
                    first_tile = False
                    ot = op.tile([128, n_sz], mybir.dt.float32, tag="o")
                    for jp in range(n_sz // pc):
                        ps = pp.tile([128, pc], mybir.dt.float32, tag="ps")
                        for jj in range(pc // fd):
                            blk = jp * (pc // fd) + jj
                            q = 32 * (blk % 4)
                            m = blk // 4
                            nc.tensor.matmul(
                                ps[:, jj * fd : (jj + 1) * fd],
                                lt[q : q + KF, i0 : i0 + 128],
                                gts[b][q : q + KF, m * fd : (m + 1) * fd],
                                start=True,
                                stop=True,
                                tile_position=(q, 0),
                            )
                        dst = ot[:, jp * pc : (jp + 1) * pc]
                        if copy_idx % 2 == 0:
                            nc.scalar.copy(dst, ps[:])
                        else:
                            nc.vector.tensor_copy(dst, ps[:])
                        copy_idx += 1
                        if first_tile:
                            # stream the very first row out per-chunk so the
                            # output DMA pipeline starts ~5us earlier
                            nc.sync.dma_start(
                                out_ext.ap()[
                                    b,
                                    it * 128 : (it + 1) * 128,
                                    jp * pc : (jp + 1) * pc,
                                ],
                                dst,
                            )
                    if not first_tile:
                        # alternate the two HWDGE rings so the SDMA engines
                        # always have two queues of work to round-robin
                        eng = nc.sync if (b * n_it + it) % 2 == 0 else nc.scalar
                        eng.dma_start(
                            out_ext.ap()[b, it * 128 : (it + 1) * 128, :], ot[:]
                        )
    nc.compile()
    _NC_CACHE[key] = nc
    return nc


def _run(points, principal_dir, linearity, **spmd_kwargs):
    b_sz, n_sz, _ = points.shape
    s_sz = n_sz // N_CORES
    G, L = _features(points, principal_dir, linearity)
    Ghi, Glo = _hilo(G)
    Lhi, Llo = _hilo(L)
    G30 = np.concatenate([Ghi, Ghi, Glo], axis=1)  # [B, 30, N]
    L30 = np.concatenate([Lhi, Llo, Lhi], axis=1)  # [B, 30, N]

    def strip4(x):  # rows 32q..32q+29 hold copies of the 30 feature rows
        sh = list(x.shape)
        sh[1] = 128
        o = np.zeros(sh, dtype=x.dtype)
        for q in range(4):
            o[:, 32 * q : 32 * q + 30] = x
        return o

    fd = min(512, n_sz // 4)
    nb = n_sz // fd
    G30 = np.ascontiguousarray(
        np.stack(
            [
                np.concatenate(
                    [G30[:, :, bb * fd : (bb + 1) * fd] for bb in range(k, nb, 4)],
                    axis=2,
                )
                for k in range(4)
            ],
            axis=1,
        )
    )  # [B, 4, 30, N/4] — fd-col blocks round-robined across the 4 strips
    nc = _build(b_sz, n_sz, s_sz)
    in_maps = [
        {
            "gfeat": G30,
            # [KF, B*S]: both batches' i-coefficients side by side -> one DMA
            "lfeat": np.ascontiguousarray(
                strip4(
                    np.concatenate(
                        [L30[b, :, c * s_sz : (c + 1) * s_sz] for b in range(b_sz)],
                        axis=1,
                    )[None]
                )[0]
            ),
        }
        for c in range(N_CORES)
    ]
    res = run_bass_kernel_spmd(nc, in_maps, core_ids=list(range(N_CORES)), **spmd_kwargs)
    out = np.concatenate([res.results[c]["out"] for c in range(N_CORES)], axis=1)
    return out, res


def kernel(points, principal_dir, linearity):
    out, _ = _run(points, principal_dir, linearity)
    return out


# revision 36
# speedup vs baseline: 1.1617x; 1.1617x over previous
"""Anisotropic distance kernel for 8 Trainium2 NeuronCores.

Math: the reference computes, per batch b and point pair (i, j):

    along  = (p_i - p_j) . t_i
    sqd    = ||p_i - p_j||^2           (gram trick)
    out    = alpha_i * max(sqd + (||t_i||^2 - 2) * along^2, 0) + beta_i * along^2

With g_i = alpha_i*(||t_i||^2 - 2) + beta_i and h_i = alpha_i this is (up to the
clamp, whose argument is nonnegative mathematically — it only clips ~1e-6
rounding noise):

    out[i, j] = g_i * along^2 + h_i * sqd

Both along^2 and sqd are quadratic polynomials in p_j, so out[i, j] is a dot
product of a 10-vector of i-coefficients with the fixed monomial basis
    G(p_j) = [p1^2, p2^2, p3^2, p1 p2, p1 p3, p2 p3, p1, p2, p3, 1].

The entire N x N map is therefore ONE matmul with contraction K=10. It runs on
the TensorEngine in bf16 with a hi/lo split (K=30) for ~fp32 accuracy at full
PE speed: sum = L_hi.G_hi + L_lo.G_hi + L_hi.G_lo (error ~2^-18 per term).
Since K=30 <= 32, the features are duplicated on two partition strips (rows
0-29 and 32-61) and consecutive matmuls alternate tile_position row groups,
so pairs of matmuls run concurrently in independent 32-row strips of the PE
array (row tiling) — the matmul supply outruns the DMA drain even cold.

Sharding: output rows i are split across the 8 cores (1024 rows/core/batch);
the 30-row j-basis G is replicated. Per core: 256 matmuls [128x512] (weights
change once per 16 matmuls), 1MB PSUM->SBUF copies alternating the scalar and
vector engines, and 4MB-per-row-block output DMAs. The kernel is bound by the
output write stream: 64MB/core through 16 SDMA engines at ~27 GB/s each
(~430 GB/s aggregate); a short junk-matmul warmup latches the PE HAM clock
gate so the matmul supply keeps the DMA queue fed from the first tile.
"""

import numpy as np
import ml_dtypes

import concourse.bass as bass  # noqa: F401  (engine namespaces live on nc)
import concourse.mybir as mybir
from concourse import bacc, tile
from concourse.bass_utils import run_bass_kernel_spmd

N_CORES = 8
KF = 30            # contraction: 10 basis monomials x 3 hi/lo cross terms
FD = 512           # matmul moving free dim (one fp32 PSUM bank)
PC = 2048          # PSUM tile columns (4 banks) per copy instruction
N_WARM = 16        # junk matmuls at t=0 to latch the PE HAM clock gate

_BF16 = ml_dtypes.bfloat16
_NC_CACHE = {}


def _features(points, principal_dir, linearity):
    """Host-side O(N) prep: the 10-row j-basis G and i-coefficients L."""
    p = points.astype(np.float64)
    t = principal_dir.astype(np.float64)
    lin = linearity.astype(np.float64)[..., 0]
    p1, p2, p3 = p[..., 0], p[..., 1], p[..., 2]
    t1, t2, t3 = t[..., 0], t[..., 1], t[..., 2]
    one = np.ones_like(p1)
    G = np.stack(
        [p1 * p1, p2 * p2, p3 * p3, p1 * p2, p1 * p3, p2 * p3, p1, p2, p3, one],
        axis=1,
    )  # [B, 10, N]
    s = (t * p).sum(-1)
    sq = (p * p).sum(-1)
    tn2 = (t * t).sum(-1)
    alpha = 2.0 * (1.0 + lin)
    beta = 0.5 * (1.0 - lin)
    g = alpha * (tn2 - 2.0) + beta
    h = alpha
    L = np.stack(
        [
            g * t1 * t1 + h,
            g * t2 * t2 + h,
            g * t3 * t3 + h,
            2.0 * g * t1 * t2,
            2.0 * g * t1 * t3,
            2.0 * g * t2 * t3,
            -2.0 * (g * s * t1 + h * p1),
            -2.0 * (g * s * t2 + h * p2),
            -2.0 * (g * s * t3 + h * p3),
            g * s * s + h * sq,
        ],
        axis=1,
    )  # [B, 10, N]
    return G, L


def _hilo(x):
    x32 = x.astype(np.float32)
    hi = x32.astype(_BF16)
    lo = (x32 - hi.astype(np.float32)).astype(_BF16)
    return hi, lo


def _build(b_sz, n_sz, s_sz):
    key = (b_sz, n_sz, s_sz)
    if key in _NC_CACHE:
        return _NC_CACHE[key]
    nc = bacc.Bacc("TRN2", debug=False, target_bir_lowering=False)
    g_ext = nc.declare_dram_parameter(
        "gfeat", [b_sz, 4, KF, n_sz // 4], mybir.dt.bfloat16, isOutput=False
    )
    l_ext = nc.declare_dram_parameter(
        "lfeat", [128, b_sz * s_sz], mybir.dt.bfloat16, isOutput=False
    )
    out_ext = nc.declare_dram_parameter(
        "out", [b_sz, s_sz, n_sz], mybir.dt.float32, isOutput=True
    )
    pc = min(PC, n_sz // 4)   # one PSUM tile spans one column-quarter (strip)
    fd = min(FD, pc)
    n_it = s_sz // 128
    copy_idx = 0
    with tile.TileContext(nc) as tc:
        with (
            tc.tile_pool(name="gpool", bufs=2) as gp,
            tc.tile_pool(name="lpool", bufs=1) as lp,
            tc.tile_pool(name="opool", bufs=4) as op,
            tc.tile_pool(name="ppool", bufs=2, space="PSUM") as pp,
        ):
            # Input loads: L first (tiny, gates everything), then the basis
            # quarter-striped: strip q (partitions 32q..32q+29) holds only
            # quarter-q's columns, so the four loads hit disjoint SDMA
            # engine sets and run concurrently. No replication, no SWDGE,
            # no PE warmup: the row-tiled matmul quads outrun the DMA drain
            # even at the cold clock.
            lt = lp.tile([128, b_sz * s_sz], mybir.dt.bfloat16)
            nc.sync.dma_start(lt[:], l_ext.ap())
            quarter = n_sz // 4
            gts = []
            for b in range(b_sz):
                gt = gp.tile([128, quarter], mybir.dt.bfloat16, tag="g")
                for k in range(4):
                    eng = nc.sync if k % 2 == 0 else nc.scalar
                    eng.dma_start(
                        gt[32 * k : 32 * k + KF, :],
                        g_ext.ap()[b, k],
                    )
                gts.append(gt)
            for b in range(b_sz):
                for it in range(n_it):
                    i0 = b * s_sz + it * 128
                    first_tile = False
                    ot = op.tile([128, n_sz], mybir.dt.float32, tag="o")
                    for jp in range(n_sz // pc):
                        ps = pp.tile([128, pc], mybir.dt.float32, tag="ps")
                        for jj in range(pc // fd):
                            blk = jp * (pc // fd) + jj
                            q = 32 * (blk % 4)
                            m = blk // 4
                            nc.tensor.matmul(
                                ps[:, jj * fd : (jj + 1) * fd],
                                lt[q : q + KF, i0 : i0 + 128],
                                gts[b][q : q + KF, m * fd : (m + 1) * fd],
                                start=True,
                                stop=True,
                                tile_position=(q, 0),
                            )
                        dst = ot[:, jp * pc : (jp + 1) * pc]
                        if copy_idx % 2 == 0:
                            nc.scalar.copy(dst, ps[:])
                        else:
                            nc.vector.tensor_copy(dst, ps[:])
                        copy_idx += 1
                        if first_tile:
                            # stream the very first row out per-chunk so the
                            # output DMA pipeline starts ~5us earlier
                            nc.sync.dma_start(
                                out_ext.ap()[
                                    b,
                                    it * 128 : (it + 1) * 128,
                                    jp * pc : (jp + 1) * pc,
                                ],
                                dst,
                            )
                    if not first_tile:
                        # split each row-block across the two HWDGE rings so
                        # the SDMA engines always have two queues of work
                        h2 = n_sz // 2
                        nc.sync.dma_start(
                            out_ext.ap()[b, it * 128 : (it + 1) * 128, :h2],
                            ot[:, :h2],
                        )
                        nc.scalar.dma_start(
                            out_ext.ap()[b, it * 128 : (it + 1) * 128, h2:],
                            ot[:, h2:],
                        )
    nc.compile()
    _NC_CACHE[key] = nc
    return nc


def _run(points, principal_dir, linearity, **spmd_kwargs):
    b_sz, n_sz, _ = points.shape
    s_sz = n_sz // N_CORES
    G, L = _features(points, principal_dir, linearity)
    Ghi, Glo = _hilo(G)
    Lhi, Llo = _hilo(L)
    G30 = np.concatenate([Ghi, Ghi, Glo], axis=1)  # [B, 30, N]
    L30 = np.concatenate([Lhi, Llo, Lhi], axis=1)  # [B, 30, N]

    def strip4(x):  # rows 32q..32q+29 hold copies of the 30 feature rows
        sh = list(x.shape)
        sh[1] = 128
        o = np.zeros(sh, dtype=x.dtype)
        for q in range(4):
            o[:, 32 * q : 32 * q + 30] = x
        return o

    fd = min(512, n_sz // 4)
    nb = n_sz // fd
    G30 = np.ascontiguousarray(
        np.stack(
            [
                np.concatenate(
                    [G30[:, :, bb * fd : (bb + 1) * fd] for bb in range(k, nb, 4)],
                    axis=2,
                )
                for k in range(4)
            ],
            axis=1,
        )
    )  # [B, 4, 30, N/4] — fd-col blocks round-robined across the 4 strips
    nc = _build(b_sz, n_sz, s_sz)
    in_maps = [
        {
            "gfeat": G30,
            # [KF, B*S]: both batches' i-coefficients side by side -> one DMA
            "lfeat": np.ascontiguousarray(
                strip4(
                    np.concatenate(
                        [L30[b, :, c * s_sz : (c + 1) * s_sz] for b in range(b_sz)],
                        axis=1,
                    )[None]
                )[0]
            ),
        }
        for c in range(N_CORES)
    ]
    res = run_bass_kernel_spmd(nc, in_maps, core_ids=list(range(N_CORES)), **spmd_kwargs)
    out = np.concatenate([res.results[c]["out"] for c in range(N_CORES)], axis=1)
    return out, res


def kernel(points, principal_dir, linearity):
    out, _ = _run(points, principal_dir, linearity)
    return out
